# revision 22
# baseline (speedup 1.0000x reference)
"""MoE AlltoAllTokenDispatcher kernel for TRN2 (8 NeuronCores).

The reference dispatcher's gather (tokens[argsort(idx)//k]) followed by
scatter-add at the same argsort permutation is an exact identity on slot
order: unpermuted[s] == tokens[s // k] for every slot s, independent of the
routing indices. The whole module therefore reduces to

    out[i] = tokens[i] * (probs[i, 0] + probs[i, 1])

a pure memory-bound row-scaling (the memory roofline is read 256 MB +
write 256 MB). Tokens are sharded across the 8 cores on the token dim
(data-parallel per the sharding hint; no all-to-all is needed since the
expert compute between dispatch and combine is identity).

Per-core kernel (Tile framework):
  - Token tile i holds tokens {16p + i} on partition p (stride-16 rows).
    With that tiling probs loads as ONE fully contiguous [128, 32] tile
    (128 B/partition) and a single strided DVE pair-add produces every
    tile's per-partition scale column — no transpose and no tiny-descriptor
    gather clogging the DMA rings (an (n p)-ordered gather would emit 2048
    8-byte descriptors and stall each SDMA engine ~15 us).
  - Loads ride the sync HWDGE ring, stores the scalar HWDGE ring, so a
    store waiting on compute never blocks a later load's dispatch.
  - First and last tiles are split in half along hidden: shorter dependency
    chains at ramp/drain. (Empirically load-bearing: without the splits the
    slot-recycle loop latency exceeds bufs x the 9.7 us/tile bandwidth
    period and the dispatch cadence drifts, costing ~36 us.)

Measured on 8 concurrent trn2 cores: ~169.6 us/core (~405 GB/s of the
~425 GB/s 16-SDMA-engine ceiling; the remainder is the fixed ~7 us
framework preamble plus ramp/drain).
"""

import numpy as np

import concourse.tile as tile
from concourse import bacc, mybir
from concourse.bass_utils import run_bass_kernel_spmd

N_TOKENS = 16384
HIDDEN = 4096
TOP_K = 2
N_CORES = 8
TOK_PER_CORE = N_TOKENS // N_CORES  # 2048
P = 128
N_TILES = TOK_PER_CORE // P  # 16
N_BUFS = 6

_nc_cache = None


def _work_items():
    """(tile_idx, col_start, ncols) per pipeline step; first and last row
    tiles are split in half along hidden."""
    items = []
    h2 = HIDDEN // 2
    for i in range(N_TILES):
        if i in (0, N_TILES - 1):
            items.append((i, 0, h2))
            items.append((i, h2, h2))
        else:
            items.append((i, 0, HIDDEN))
    return items


def _build_nc():
    nc = bacc.Bacc(
        "TRN2", target_bir_lowering=False, debug=False, num_devices=N_CORES
    )
    tokens = nc.dram_tensor(
        "tokens", [TOK_PER_CORE, HIDDEN], mybir.dt.float32, kind="ExternalInput"
    ).ap()
    probs = nc.dram_tensor(
        "probs", [TOK_PER_CORE, TOP_K], mybir.dt.float32, kind="ExternalInput"
    ).ap()
    out = nc.dram_tensor(
        "out", [TOK_PER_CORE, HIDDEN], mybir.dt.float32, kind="ExternalOutput"
    ).ap()
    # tile i, partition p  <->  token row 16p + i
    tok_t = tokens.rearrange("(p n) m -> n p m", n=N_TILES)
    out_t = out.rearrange("(p n) m -> n p m", n=N_TILES)

    with tile.TileContext(nc) as tc:
        with (
            tc.tile_pool(name="tok", bufs=N_BUFS) as tok_pool,
            tc.tile_pool(name="pr", bufs=1) as pr_pool,
            tc.tile_pool(name="sc", bufs=1) as sc_pool,
        ):
            # pt[p, (j k)] <- probs[16p+j, k]: one contiguous DMA, then
            # st[p, j] = pt[p, 2j] + pt[p, 2j+1]: one strided DVE add.
            pt = pr_pool.tile([P, N_TILES * TOP_K], mybir.dt.float32)
            st = sc_pool.tile([P, N_TILES], mybir.dt.float32)
            nc.scalar.dma_start(
                out=pt[:],
                in_=probs.rearrange("(p j) k -> p (j k)", j=N_TILES),
            )
            pt3 = pt[:].rearrange("p (j k) -> p j k", k=TOP_K)
            nc.vector.tensor_add(
                st[:].rearrange("p (j o) -> p j o", o=1),
                pt3[:, :, 0:1],
                pt3[:, :, 1:2],
            )

            for i, c0, ncols in _work_items():
                tt = tok_pool.tile([P, ncols], mybir.dt.float32, tag="tok")
                nc.sync.dma_start(
                    out=tt[:, :ncols], in_=tok_t[i, :, c0 : c0 + ncols]
                )
                nc.vector.tensor_scalar_mul(
                    tt[:, :ncols], tt[:, :ncols], st[:, i : i + 1]
                )
                nc.scalar.dma_start(
                    out=out_t[i, :, c0 : c0 + ncols], in_=tt[:, :ncols]
                )
    nc.compile()
    return nc


def kernel(tokens, probs, indices=None, **_unused):
    global _nc_cache
    tokens = np.ascontiguousarray(np.asarray(tokens, dtype=np.float32))
    probs = np.ascontiguousarray(np.asarray(probs, dtype=np.float32))
    assert tokens.shape == (N_TOKENS, HIDDEN), tokens.shape
    assert probs.shape == (N_TOKENS, TOP_K), probs.shape

    if _nc_cache is None:
        _nc_cache = _build_nc()

    in_maps = [
        {
            "tokens": tokens[c * TOK_PER_CORE : (c + 1) * TOK_PER_CORE],
            "probs": probs[c * TOK_PER_CORE : (c + 1) * TOK_PER_CORE],
        }
        for c in range(N_CORES)
    ]
    res = run_bass_kernel_spmd(
        _nc_cache, in_maps, core_ids=list(range(N_CORES))
    )
    return np.concatenate([res.results[c]["out"] for c in range(N_CORES)], axis=0)
